# revision 106
# baseline (speedup 1.0000x reference)
"""Causal self-attention (B=1, T=2048, D=1024, H=8, hd=128) on 8 trn2 cores.

Sharding: tensor-parallel over heads -- one head per core. Each core computes
its head's qkv projection, rms-norm+rotary, causal attention, and the c_proj
partial product for its head; the host sums the 8 partial [D, T] outputs.

81.1us baseline -> 65.9us, via engine balance + phase overlap (end-to-end
rel err ~1.1e-3 on hardware vs the 2e-2 gate):
  - qkv projection in fp8(e4m3) DoubleRow (x and 64*W split into hi/lo fp8
    planes; 3 of 4 cross terms kept). Everything downstream runs f16.
  - Phase overlap: score matmuls for t-blocks 0/1 are emitted inside phase 1
    (they only need transpose groups <= tj), filling PE's x-DMA wait gaps;
    4 pT buffers keep all four t-blocks' probabilities live.
  - ACT keeps only: Exp (paired 2-bank score tiles for blocks 2/3, single
    banks with 2 bufs for the overlapped early blocks), the qkv drains, and
    half the c_proj/transpose drains. One act-table load total: rsqrt for
    the rms scale is a DVE int-bit-trick seed + 2 Newton steps, so Sqrt
    never touches ACT.
  - sumsq fused into the qkv drain path as DVE scalar_tensor_tensor with
    accum_out; v-drain fused with the ve add the same way; q/k normalized
    in place with per-partition tensor_scalar (diag matmul trick dropped).
  - dn+yT share one PSUM bank per 256-wide half t-block (one accumulation
    group; first element write overwrites, later ones add), double-buffered;
    for t-blocks >= 1 adjacent full probability blocks are pre-summed on
    DVE so the softmax denominator needs one PE matmul per pair.
  - GPSIMD (Pool) cannot touch PSUM on trn2: it takes only SBUF work
    (rotary half, causal masks, Newton muls).
  - 40 warm-up matmuls on a zeroed tile anchor the PE p-state ramp during
    the initial DMA gate, so real matmuls run at 2.4GHz from the start.
  - Tail: the last t-block's av/cproj run in 256-wide halves, its c_proj
    PSUM tiles cycle three pools (8 banks), stores split per 4 c_proj rows.
"""

import numpy as np

B, T, D = 1, 2048, 1024
H, HD = 8, 128
SCALE = 0.12
NCORES = 8
NT = T // 128      # 16 token tiles
NCH = D // 128     # 8 contraction chunks
NPAIR = NCH // 2   # 4 DoubleRow chunk pairs
NTJ = 4            # attention t-blocks
TJ = T // NTJ      # 512
EPS = float(np.finfo(np.float32).eps)
WS = 64.0          # fp8 weight prescale
ESH = 4.5          # exp shift

_CACHE = {}


def _bcast(ap, n):
    """Broadcast a [..., 1] AP to [..., n] via a step-0 trailing dim."""
    try:
        return ap.to_broadcast(list(ap.shape[:-1]) + [n])
    except Exception:
        import concourse.bass as bass
        return bass.AP(tensor=ap.tensor, offset=ap.offset,
                       ap=list(ap.ap[:-1]) + [[0, n]])


def _bcast_mid(ap, n):
    """Insert a step-0 middle dim: [p, f] -> [p, n, f]."""
    import concourse.bass as bass
    return bass.AP(tensor=ap.tensor, offset=ap.offset,
                   ap=[list(ap.ap[0]), [0, n], list(ap.ap[1])])


def _view(tile_ap, start, dims):
    """View into a flat [128, N] AP at elem offset `start` with free dims
    [(stride, count), ...]."""
    import concourse.bass as bass
    return bass.AP(tensor=tile_ap.tensor, offset=tile_ap.offset + start,
                   ap=[list(tile_ap.ap[0])] + [list(d) for d in dims])


def _drive(*gens):
    """Round-robin drive generators to completion (interleaves PE work)."""
    gens = [g for g in gens if g is not None]
    while gens:
        nxt = []
        for g in gens:
            try:
                next(g)
                nxt.append(g)
            except StopIteration:
                pass
        gens = nxt


def _build_program():
    if "nc" in _CACHE:
        return _CACHE["nc"]

    import concourse.bacc as bacc
    import concourse.tile as tile
    import concourse.mybir as mybir

    f32 = mybir.dt.float32
    f16 = mybir.dt.float16
    fp8 = mybir.dt.float8e4
    AF = mybir.ActivationFunctionType
    ALU = mybir.AluOpType
    DR = mybir.MatmulPerfMode.DoubleRow

    nc = bacc.Bacc("TRN2", target_bir_lowering=False, debug=False)

    # x planes: [batch of 4 tiles, c-in-part 128, tile, chunk, token]
    xh_d = nc.dram_tensor("xh", [4, 128, 4, NCH, 128], fp8,
                          kind="ExternalInput")
    xl_d = nc.dram_tensor("xl", [4, 128, 4, NCH, 128], fp8,
                          kind="ExternalInput")
    wh_d = nc.dram_tensor("wh", [128, NCH, 3 * HD], fp8, kind="ExternalInput")
    wl_d = nc.dram_tensor("wl", [128, NCH, 3 * HD], fp8, kind="ExternalInput")
    ve_d = nc.dram_tensor("veN", [128, NT, HD], f16, kind="ExternalInput")
    cw_d = nc.dram_tensor("cwT", [HD, D], f16, kind="ExternalInput")
    # packed consts per partition: cos(512) | sin(512) | tri(128) | idn(128)
    cst_d = nc.dram_tensor("cst", [128, 1280], f16, kind="ExternalInput")
    out_d = nc.dram_tensor("outT", [D, T], f16, kind="ExternalOutput")

    with tile.TileContext(nc) as tc:
        with tc.tile_pool(name="const", bufs=1) as cpool, \
             tc.tile_pool(name="work", bufs=1) as wpool, \
             tc.tile_pool(name="xs", bufs=1) as xpool:
            # ---- resident inputs ----
            wh_sb = cpool.tile([128, NCH, 3 * HD], fp8)
            wl_sb = cpool.tile([128, NCH, 3 * HD], fp8)
            ve_sb = cpool.tile([128, NT, HD], f16)
            cw_sb = cpool.tile([HD, D], f16)
            cst_sb = cpool.tile([128, 1280], f16)
            ones_sb = cpool.tile([128, 128], f16)
            bexp_sb = cpool.tile([128, 1], f32)   # exp shift bias

            cstf = cst_sb[:]

            def cos_g(g):     # [128, 4, 32] for tile group g
                return _view(cstf, 128 * g, [[32, 4], [1, 32]])

            def sin_g(g):
                return _view(cstf, 512 + 128 * g, [[32, 4], [1, 32]])

            tri_v = _view(cstf, 1024, [[1, 128]])
            idn_v = _view(cstf, 1152, [[1, 128]])

            wu_sb = cpool.tile([128, 128], f16)   # PE warm-up operand
            nc.vector.memset(ones_sb[:], 1.0)
            nc.vector.memset(bexp_sb[:], -ESH)
            nc.vector.memset(wu_sb[:], 0.0)

            # ---- working buffers ----
            qkv = wpool.tile([128, NT, 3 * HD], f16)      # natural qkv
            sqs = wpool.tile([128, HD], f16)              # square scratch out
            rs = wpool.tile([128, NT, 2], f32)            # rms scalars q,k
            rx = wpool.tile([128, NT, 2], f32)            # rsqrt arg scratch
            rt = wpool.tile([128, NT, 2], f32)            # newton scratch
            qT_sb = wpool.tile([128, T], f16)             # q-hat.T [d, t]
            kT_sb = wpool.tile([128, NT, 128], f16)       # k-hat.T [d, si, s']
            pT_bufs = [wpool.tile([128, NT, TJ], f16, tag=f"pT{i}",
                                  name=f"pT{i}") for i in range(4)]

            # ============ phase 1: qkv (fp8 DoubleRow) + rms + rotary ======
            pools = {}
            if True:

                def dma_order_for_group(g):
                    # interleave remaining loads so x batches stay critical
                    if g < 3:
                        gs4 = slice(4 * (g + 1), 4 * (g + 2))
                        nc.sync.dma_start(ve_sb[:, gs4], ve_d[:, gs4])
                    if g == 1:
                        nc.sync.dma_start(cst_sb[:], cst_d[:])
                    elif g == 2:
                        nc.sync.dma_start(cw_sb[:], cw_d[:])

                def emit_qkv_mms(g, ps_tiles, xh_b, xl_b, term_major):
                    """24 DR matmuls per tile; term-major runs the 3 planes
                    as waves across all 4 tiles (x/w DMA arrival order)."""
                    terms = ((xh_b, wh_sb, NPAIR), (xl_b, wh_sb, NPAIR),
                             (xh_b, wl_sb, NPAIR))
                    i_mm = [0] * 4
                    n_mm = sum(2 * t[2] for t in terms)

                    def one(j, t_idx, P, half):
                        xa, wa, _ = terms[t_idx]
                        cs = slice(2 * P, 2 * P + 2)
                        hs2 = slice(192 * half, 192 * (half + 1))
                        nc.tensor.matmul(
                            ps_tiles[j][:, hs2],
                            xa[:, j, cs, :],
                            wa[:, cs, hs2],
                            start=(i_mm[j] == 0),
                            stop=(i_mm[j] == n_mm - 1),
                            perf_mode=DR,
                        )
                        i_mm[j] += 1

                    if term_major:
                        for t_idx in range(3):
                            for j in range(4):
                                for P in range(terms[t_idx][2]):
                                    for half in range(2):
                                        one(j, t_idx, P, half)
                    else:
                        for j in range(4):
                            for t_idx in range(3):
                                for P in range(terms[t_idx][2]):
                                    for half in range(2):
                                        one(j, t_idx, P, half)

                def emit_qkv_group(g):
                    gs = slice(4 * g, 4 * (g + 1))
                    xh_b = xpool.tile([128, 4, NCH, 128], fp8,
                                      tag=f"xh{g % 2}", name=f"xh_b{g}")
                    xl_b = xpool.tile([128, 4, NCH, 128], fp8,
                                      tag=f"xl{g % 2}", name=f"xl_b{g}")
                    if g == 0:
                        # critical order: first matmul wave needs xh+wh only
                        nc.sync.dma_start(xh_b[:], xh_d[g])
                        nc.sync.dma_start(wh_sb[:], wh_d[:])
                        nc.sync.dma_start(xl_b[:], xl_d[g])
                        nc.sync.dma_start(wl_sb[:], wl_d[:])
                        nc.sync.dma_start(ve_sb[:, 0:4], ve_d[:, 0:4])
                    else:
                        nc.sync.dma_start(xh_b[:], xh_d[g])
                        nc.sync.dma_start(xl_b[:], xl_d[g])
                    if g == 0:
                        # warm-up mms against a zero tile: anchor the PE
                        # p-state ramp during the initial x/w DMA gate so
                        # real matmuls run at full clock from the start
                        wps = pools["pq"].tile([128, 128], f32, tag="qkvp",
                                               name="wu_ps")
                        for _ in range(40):
                            nc.tensor.matmul(wps[:], wu_sb[:], wu_sb[:],
                                             start=True, stop=True)
                    ps_tiles = [pools["pq"].tile([128, 3 * HD], f32, tag="qkvp",
                                        name=f"ps{g}_{j}") for j in range(4)]
                    emit_qkv_mms(g, ps_tiles, xh_b, xl_b, term_major=(g == 0))
                    dma_order_for_group(g)
                    for j in range(4):
                        ti = 4 * g + j
                        ps = ps_tiles[j]
                        # qk drain f32->f16 with 1/WS descale (ACT)
                        nc.scalar.activation(qkv[:, ti, 0:2 * HD],
                                             ps[:, 0:2 * HD], AF.Copy,
                                             bias=0.0, scale=1.0 / WS)
                        # fused square+sum on DVE (STT with accum_out):
                        # rs[:,ti,c] = sum(q*q), f16 2x mode
                        nc.vector.scalar_tensor_tensor(
                            sqs[:], qkv[:, ti, 0:HD], 1.0,
                            qkv[:, ti, 0:HD], op0=ALU.bypass, op1=ALU.mult,
                            accum_out=rs[:, ti, 0:1])
                        nc.vector.scalar_tensor_tensor(
                            sqs[:], qkv[:, ti, HD:2 * HD], 1.0,
                            qkv[:, ti, HD:2 * HD], op0=ALU.bypass,
                            op1=ALU.mult, accum_out=rs[:, ti, 1:2])
                    # v drain fused with ve add (ve prescaled by lambda1;
                    # lambda0 folded into W plane scale): v = ps_v/WS + ve.
                    # Emitted promptly so the qkv PSUM banks release early
                    # (phase-2 score tiles reuse them).
                    for j in range(4):
                        ti = 4 * g + j
                        nc.vector.scalar_tensor_tensor(
                            qkv[:, ti, 2 * HD:], ps_tiles[j][:, 2 * HD:],
                            1.0 / WS, ve_sb[:, ti, :],
                            op0=ALU.mult, op1=ALU.add)
                    # rq = SCALE/sqrt(ms+eps), rk = 1/sqrt(ms+eps), computed
                    # as rsqrt on DVE (bit-trick seed + 2 Newton steps) so
                    # ACT never needs the sqrt table (one act-table load for
                    # the whole program).
                    xq = rx[:, gs, 0]
                    xk = rx[:, gs, 1]
                    nc.vector.tensor_scalar(xq, rs[:, gs, 0],
                                            1.0 / (HD * SCALE * SCALE),
                                            EPS / (SCALE * SCALE),
                                            op0=ALU.mult, op1=ALU.add)
                    nc.vector.tensor_scalar(xk, rs[:, gs, 1], 1.0 / HD,
                                            EPS, op0=ALU.mult, op1=ALU.add)
                    xz = rx[:, gs, :]
                    z = rs[:, gs, :]
                    t2 = rt[:, gs, :]
                    i32 = mybir.dt.int32
                    # seed: y0 = bits(0x5f3759df - (bits(x) >> 1));
                    # tensor_scalar is DVE-only, TT muls ride on Pool
                    nc.vector.tensor_scalar(z.bitcast(i32), xz.bitcast(i32),
                                            1, None,
                                            op0=ALU.arith_shift_right)
                    nc.vector.tensor_scalar(z.bitcast(i32), z.bitcast(i32),
                                            -1, None,
                                            op0=ALU.bitwise_xor)
                    nc.vector.tensor_scalar(z.bitcast(i32), z.bitcast(i32),
                                            0x5f3759df + 1, None,
                                            op0=ALU.add)
                    for _ in range(2):   # newton: y *= 1.5 - 0.5*x*y*y
                        nc.vector.tensor_mul(t2, z, z)
                        nc.vector.tensor_mul(t2, t2, xz)
                        nc.vector.tensor_scalar(t2, t2, -0.5, 1.5,
                                                op0=ALU.mult, op1=ALU.add)
                        nc.vector.tensor_mul(z, z, t2)

                def emit_tr_group(g):
                    hs = slice(4 * g, 4 * (g + 1))
                    for base in (0, HD):
                        eng = nc.vector if base == 0 else nc.gpsimd
                        x1 = qkv[:, hs, base + 0:base + 32]
                        x2 = qkv[:, hs, base + 64:base + 96]
                        t1 = wpool.tile([128, 4, 32], f16, tag=f"rot1{base}",
                                        name=f"t1_{base}")
                        t2 = wpool.tile([128, 4, 32], f16, tag=f"rot2{base}",
                                        name=f"t2_{base}")
                        t3 = wpool.tile([128, 4, 32], f16, tag=f"rot3{base}",
                                        name=f"t3_{base}")
                        t4 = wpool.tile([128, 4, 32], f16, tag=f"rot4{base}",
                                        name=f"t4_{base}")
                        eng.tensor_mul(t1[:], x1, cos_g(g))
                        eng.tensor_mul(t2[:], x2, sin_g(g))
                        eng.tensor_mul(t3[:], x2, cos_g(g))
                        eng.tensor_mul(t4[:], x1, sin_g(g))
                        eng.tensor_add(x1, t1[:], t2[:])
                        eng.tensor_sub(x2, t3[:], t4[:])
                    # normalize in place: q *= rq[t], k *= rk[t] (per-token
                    # scalar AP exempt from the DVE 2x-mode dtype check)
                    for j in range(4):
                        ti = 4 * g + j
                        nc.vector.tensor_scalar_mul(qkv[:, ti, 0:HD],
                                                    qkv[:, ti, 0:HD],
                                                    rs[:, ti, 0:1])
                        nc.vector.tensor_scalar_mul(qkv[:, ti, HD:2 * HD],
                                                    qkv[:, ti, HD:2 * HD],
                                                    rs[:, ti, 1:2])
                    # plain transposes (PE), drains on Pool
                    for base, isq in ((0, True), (HD, False)):
                        tp = pools["ptr"].tile([128, 4, 128], f32, tag="trp")
                        for j in range(4):
                            ti = 4 * g + j
                            nc.tensor.matmul(
                                tp[:, j, :],
                                qkv[:, ti, base:base + HD],
                                idn_v, start=True, stop=True)
                        dview = qT_sb[:, 512 * g:512 * (g + 1)] if isq \
                            else kT_sb[:, 4 * g:4 * (g + 1), :] \
                            .rearrange("p a b -> p (a b)")
                        tpf = tp[:].rearrange("p a b -> p (a b)")
                        # PSUM drains are ACT/DVE only (gpsimd can't)
                        if isq == (g % 2 == 0):
                            nc.scalar.copy(dview, tpf)
                        else:
                            nc.vector.tensor_copy(dview, tpf)

                def act_copy(dst, src):
                    nc.scalar.copy(dst, src)

                def dve_copy(dst, src):
                    nc.vector.tensor_copy(dst, src)

                # PSUM drains are ACT/DVE only (gpsimd can't touch PSUM).
                # Early c_proj blocks drain DVE-heavy (ACT busy with exps);
                # late blocks ACT-heavy (exps done by then).
                drain_early = [dve_copy, act_copy, dve_copy, act_copy,
                               dve_copy, act_copy, dve_copy, act_copy]
                drain_late = [act_copy, dve_copy, act_copy, act_copy,
                              dve_copy, act_copy, act_copy, dve_copy]

                def gen_scores(tj, psc, pw=2):
                    pT_sb = pT_bufs[tj]
                    t0 = TJ * tj
                    n_act = 4 * (tj + 1)
                    for p in range(n_act // pw):
                        sc2 = psc.tile([128, pw, TJ], f32, tag="scp",
                                       name=f"sc2_{tj}_{p}")
                        # diagonal blocks compute their full row too (the
                        # sub-diagonal part is finite garbage, never read by
                        # dn/yT) so the paired exp reads fully-written PSUM
                        for b in range(pw):
                            si = pw * p + b
                            nc.tensor.matmul(
                                sc2[:, b, :], kT_sb[:, si, :],
                                qT_sb[:, t0:t0 + TJ],
                                start=True, stop=True)
                            yield
                        # paired exp over both banks (garbage regions of
                        # diagonal blocks are exp'd too but never read)
                        nc.scalar.activation(
                            pT_sb[:, pw * p:pw * p + pw, :], sc2[:], AF.Exp,
                            bias=bexp_sb[:])
                    # merged diagonal triangle mask (DVE)
                    import concourse.bass as bass
                    base = pT_sb[:, 4 * tj, 0:128]
                    mview = bass.AP(
                        tensor=base.tensor, offset=base.offset,
                        ap=[list(base.ap[0]), [TJ + 128, 4], [1, 128]])
                    tri_b = _bcast_mid(tri_v, 4)
                    # SBUF-only -> Pool (keeps ACT/DVE free for drains)
                    nc.gpsimd.tensor_tensor(mview, mview, tri_b, op=ALU.mult)

                def gen_av(tj, out_yh, pav, fin):
                    """dn+yT per 256-wide half, sharing one PSUM bank
                    ([128, 2, 256] f32), double-buffered across halves.
                    For the two big t-blocks, adjacent full probability
                    blocks are pre-summed on DVE so the denominator needs
                    one PE matmul per pair."""
                    pT_sb = pT_bufs[tj]
                    n_act = 4 * (tj + 1)
                    npair = (4 * tj) // 2 if tj >= 1 else 0
                    if npair:
                        pS = wpool.tile([128, 6, TJ], f16, tag="pSum",
                                        name=f"pS_{tj}")
                        for p in range(npair):
                            nc.vector.tensor_add(pS[:, p, :],
                                                 pT_sb[:, 2 * p, :],
                                                 pT_sb[:, 2 * p + 1, :])
                    yh = fin.tile([128, TJ], f16, tag=f"yh{tj % 2}",
                                  name=f"yh_{tj}")
                    chunks = ((0, 256), (256, 256))
                    for h, (t0h, tl) in enumerate(chunks):
                        dy = pav.tile([128, 2, 256], f32, tag="dy",
                                      name=f"dy_{tj}_{h}")
                        act = [si for si in range(n_act)
                               if (128 * (si - 4 * tj)
                                   if si > 4 * tj else 0) < t0h + tl]
                        # dn+yT share one PSUM bank: ONE accumulation group
                        # (start on the first mm, stop on the last; first
                        # write per element overwrites, later ones add).
                        # dn ops: paired sums for si < 2*npair, then singles.
                        dn_ops = [("p", p) for p in range(npair)] + \
                                 [("s", si) for si in act
                                  if si >= 2 * npair]
                        n_ops = len(dn_ops) + len(act)
                        iop = 0
                        for kind, ix in dn_ops:
                            if kind == "p":
                                srcap = pS[:, ix, t0h:t0h + tl]
                                s0 = 0
                            else:
                                o = ix - 4 * tj
                                off = 128 * o if o > 0 else 0
                                s0 = max(off - t0h, 0)
                                srcap = pT_sb[:, ix, t0h + s0:t0h + tl]
                            nc.tensor.matmul(
                                dy[:, 0, s0:tl], ones_sb[:], srcap,
                                start=(iop == 0), stop=False)
                            iop += 1
                            yield
                        for idx, si in enumerate(act):
                            o = si - 4 * tj
                            off = 128 * o if o > 0 else 0
                            s0 = max(off - t0h, 0)
                            nc.tensor.matmul(
                                dy[:, 1, s0:tl], qkv[:, si, 2 * HD:],
                                pT_sb[:, si, t0h + s0:t0h + tl],
                                start=False, stop=(idx == len(act) - 1))
                            yield
                        rdn = fin.tile([128, 256], f32, tag=f"rdn{h % 2}",
                                       name=f"rdn_{tj}_{h}")
                        nc.vector.reciprocal(rdn[:, 0:tl], dy[:, 0, 0:tl])
                        nc.vector.tensor_mul(yh[:, t0h:t0h + tl],
                                             dy[:, 1, 0:tl], rdn[:, 0:tl])
                    out_yh.append(yh)

                def gen_cproj(tj, yh, po, stp, po2=None, po3=None):
                    import concourse.bass as bass
                    st = stp.tile([128, 8, TJ], f16, tag=f"st{tj % 2}",
                                  name=f"st_{tj}")
                    halves = ((0, 256), (256, 256)) \
                        if tj == 3 else ((0, TJ),)
                    e_per = 4
                    for (t0h, tl) in halves:
                        t0 = TJ * tj + t0h
                        for e in range(8):
                            # tail cycles three pools (psc2 is idle by then)
                            # so the mms aren't PSUM-bank starved
                            if po3 is not None and e % 3 == 2:
                                pot = po3.tile([128, 2, tl], f32, tag="scp",
                                               name=f"pot_{tj}_{t0h}_{e}")
                                pot = pot[:, 0, :]
                            elif po2 is not None and e % 3 == 1:
                                pot = po2.tile([128, tl], f32, tag="dy",
                                               name=f"pot_{tj}_{t0h}_{e}")
                                pot = pot[:]
                            else:
                                pot = po.tile([128, tl], f32, tag="pot",
                                              name=f"pot_{tj}_{t0h}_{e}")
                            pap = pot if not hasattr(pot, 'tile') else pot
                            try:
                                pap = pot[:]
                            except Exception:
                                pap = pot
                            nc.tensor.matmul(
                                pap, cw_sb[:, 128 * e:128 * (e + 1)],
                                yh[:, t0h:t0h + tl],
                                start=True, stop=True)
                            yield
                            (drain_early if tj < 2 else drain_late)[e](st[:, e, t0h:t0h + tl], pap)
                            if e % e_per == e_per - 1:
                                e0 = e - e_per + 1
                                dst = out_d[0:128, t0:t0 + tl]
                                dstb = bass.AP(
                                    tensor=dst.tensor,
                                    offset=dst.offset + 128 * e0 * T,
                                    ap=[list(dst.ap[0]), [128 * T, e_per],
                                        list(dst.ap[1])])
                                nc.sync.dma_start(
                                    dstb, st[:, e0:e0 + e_per, t0h:t0h + tl])

                # ==== emission ====
                # phase 1 with scores(0..1) overlapped: scores(tj) only
                # needs transpose groups <= tj, so they fill PE's DMA-wait
                # gaps. PSUM: pq(4) + ptr(2) + psc1(2 banks, bufs=1) = 8.
                with tc.tile_pool(name="ps_qkv", bufs=4, space="PSUM") as pq_, \
                     tc.tile_pool(name="ps_tr", bufs=2, space="PSUM") as ptr_, \
                     tc.tile_pool(name="ps_sc1", bufs=2, space="PSUM") as psc1:
                    pools["pq"] = pq_
                    pools["ptr"] = ptr_
                    for g in range(4):
                        emit_qkv_group(g)
                        if g >= 1:
                            emit_tr_group(g - 1)
                            if g - 1 <= 1:
                                _drive(gen_scores(g - 1, psc1, pw=1))
                    emit_tr_group(3)

                # phase 2: scores(2..3) + av/cproj; the freed phase-1 banks
                # become psc2(4) + pdn(1) + py(1) + po(2) = 8.
                with tc.tile_pool(name="ps_sc2", bufs=1, space="PSUM") as psc2, \
                     tc.tile_pool(name="ps_av", bufs=2, space="PSUM") as pav, \
                     tc.tile_pool(name="ps_o", bufs=2, space="PSUM") as po, \
                     tc.tile_pool(name="fin", bufs=2) as fin, \
                     tc.tile_pool(name="stp", bufs=1) as stp:
                    # tj=0 (the smallest attention block) is processed LAST
                    # so the terminal av->yh->cproj->store chain is short
                    yhs = {}

                    def av(tj):
                        out = []
                        g = gen_av(tj, out, pav, fin)
                        yhs[tj] = lambda: out[0]
                        return g

                    def cproj(tj, **kw):
                        return gen_cproj(tj, yhs[tj](), po, stp, **kw)

                    _drive(gen_scores(2, psc2, pw=4), av(0))
                    _drive(gen_scores(3, psc2, pw=4), av(1), cproj(0))
                    _drive(av(2), cproj(1))
                    _drive(av(3), cproj(2))
                    _drive(cproj(3, po2=pav, po3=psc2))

    nc.compile()
    _CACHE["nc"] = nc
    return nc


def _host_inputs(x, ve, qkv_w, lambdas, c_proj_w):
    """Build the 8 per-core input maps (layout transforms only)."""
    import ml_dtypes
    f16 = ml_dtypes.float16 if hasattr(ml_dtypes, "float16") else np.float16
    e4 = ml_dtypes.float8_e4m3
    x = np.asarray(x, np.float32)
    ve = np.asarray(ve, np.float32)
    qkv_w = np.asarray(qkv_w, np.float32)
    lambdas = np.asarray(lambdas, np.float32)
    c_proj_w = np.asarray(c_proj_w, np.float32)

    # x planes: [4 batch, 128 cin, 4 tile, NCH, 128 tok]
    xr = x[0].reshape(NT, 128, NCH, 128).transpose(0, 3, 2, 1)
    xr = xr.reshape(4, 4, 128, NCH, 128).transpose(0, 2, 1, 3, 4)
    xh = xr.astype(e4)
    xl = (xr - xh.astype(np.float32)).astype(e4)
    xh = np.ascontiguousarray(xh)
    xl = np.ascontiguousarray(xl)

    freq = (1.0 / 1024.0) ** np.linspace(0.0, 1.0, HD // 4, dtype=np.float32)
    theta = np.arange(T, dtype=np.float32)[:, None] * freq[None, :]  # [T, 32]
    cosT = np.cos(theta).astype(f16).reshape(NT, 128, 32).transpose(1, 0, 2)
    sinT = np.sin(theta).astype(f16).reshape(NT, 128, 32).transpose(1, 0, 2)
    tri = (np.arange(128)[None, :] >= np.arange(128)[:, None]).astype(f16)
    idn = np.eye(128, dtype=np.float32).astype(f16)
    cst = np.ascontiguousarray(np.concatenate([
        cosT.reshape(128, 512), sinT.reshape(128, 512), tri, idn,
    ], axis=1))  # [128, 1280]

    lam0, lam1 = float(lambdas[0]), float(lambdas[1])
    wscale = np.concatenate([np.full(2 * HD, WS, np.float32),
                             np.full(HD, WS * lam0, np.float32)])

    in_maps = []
    for h in range(NCORES):
        sl = slice(128 * h, 128 * (h + 1))
        # W planes: [128 cin, NCH, 3*HD], prescaled
        whd = qkv_w[:, sl, :]                          # [3, 128, 1024]
        wt = whd.transpose(2, 0, 1).reshape(D, 3 * HD)  # [cin-full, 384]
        wt = wt * wscale[None, :]
        wt = wt.reshape(NCH, 128, 3 * HD).transpose(1, 0, 2)  # [128, NCH, 384]
        wh = wt.astype(e4)
        wl = (wt - wh.astype(np.float32)).astype(e4)
        wh = np.ascontiguousarray(wh)
        wl = np.ascontiguousarray(wl)
        veh = np.ascontiguousarray(
            (lam1 * ve[0, :, sl]).reshape(NT, 128, HD)
            .transpose(1, 0, 2).astype(f16))
        cwh = np.ascontiguousarray(c_proj_w[:, sl].T.astype(f16))  # [128, 1024]
        in_maps.append({
            "xh": xh, "xl": xl, "wh": wh, "wl": wl, "veN": veh, "cwT": cwh,
            "cst": cst,
        })
    return in_maps


def run(x, ve, qkv_w, lambdas, c_proj_w, trace=False):
    from concourse.bass_utils import run_bass_kernel_spmd

    nc = _build_program()
    in_maps = _host_inputs(x, ve, qkv_w, lambdas, c_proj_w)
    res = run_bass_kernel_spmd(
        nc, in_maps, core_ids=list(range(NCORES)), trace=trace)
    acc = np.zeros((D, T), np.float64)
    for r in res.results:
        acc += r["outT"].astype(np.float64)
    out = acc.astype(np.float32).T.reshape(B, T, D)
    return out, res


def kernel(x, ve, qkv_w, lambdas, c_proj_w):
    out, _ = run(x, ve, qkv_w, lambdas, c_proj_w, trace=False)
    return out


# revision 107
# speedup vs baseline: 1.0326x; 1.0326x over previous
"""Causal self-attention (B=1, T=2048, D=1024, H=8, hd=128) on 8 trn2 cores.

Sharding: tensor-parallel over heads -- one head per core. Each core computes
its head's qkv projection, rms-norm+rotary, causal attention, and the c_proj
partial product for its head; the host sums the 8 partial [D, T] outputs.

81.1us baseline -> 65.9us, via engine balance + phase overlap (end-to-end
rel err ~1.1e-3 on hardware vs the 2e-2 gate):
  - qkv projection in fp8(e4m3) DoubleRow (x and 64*W split into hi/lo fp8
    planes; 3 of 4 cross terms kept). Everything downstream runs f16.
  - Phase overlap: score matmuls for t-blocks 0/1 are emitted inside phase 1
    (they only need transpose groups <= tj), filling PE's x-DMA wait gaps;
    4 pT buffers keep all four t-blocks' probabilities live.
  - ACT keeps only: Exp (paired 2-bank score tiles for blocks 2/3, single
    banks with 2 bufs for the overlapped early blocks), the qkv drains, and
    half the c_proj/transpose drains. One act-table load total: rsqrt for
    the rms scale is a DVE int-bit-trick seed + 2 Newton steps, so Sqrt
    never touches ACT.
  - sumsq fused into the qkv drain path as DVE scalar_tensor_tensor with
    accum_out; v-drain fused with the ve add the same way; q/k normalized
    in place with per-partition tensor_scalar (diag matmul trick dropped).
  - dn+yT share one PSUM bank per 256-wide half t-block (one accumulation
    group; first element write overwrites, later ones add), double-buffered;
    for t-blocks >= 1 adjacent full probability blocks are pre-summed on
    DVE so the softmax denominator needs one PE matmul per pair.
  - GPSIMD (Pool) cannot touch PSUM on trn2: it takes only SBUF work
    (rotary half, causal masks, Newton muls).
  - 40 warm-up matmuls on a zeroed tile anchor the PE p-state ramp during
    the initial DMA gate, so real matmuls run at 2.4GHz from the start.
  - Tail: the last t-block's av/cproj run in 256-wide halves, its c_proj
    PSUM tiles cycle three pools (8 banks), stores split per 4 c_proj rows.
"""

import numpy as np

B, T, D = 1, 2048, 1024
H, HD = 8, 128
SCALE = 0.12
NCORES = 8
NT = T // 128      # 16 token tiles
NCH = D // 128     # 8 contraction chunks
NPAIR = NCH // 2   # 4 DoubleRow chunk pairs
NTJ = 4            # attention t-blocks
TJ = T // NTJ      # 512
EPS = float(np.finfo(np.float32).eps)
WS = 64.0          # fp8 weight prescale
ESH = 4.5          # exp shift

_CACHE = {}


def _bcast(ap, n):
    """Broadcast a [..., 1] AP to [..., n] via a step-0 trailing dim."""
    try:
        return ap.to_broadcast(list(ap.shape[:-1]) + [n])
    except Exception:
        import concourse.bass as bass
        return bass.AP(tensor=ap.tensor, offset=ap.offset,
                       ap=list(ap.ap[:-1]) + [[0, n]])


def _bcast_mid(ap, n):
    """Insert a step-0 middle dim: [p, f] -> [p, n, f]."""
    import concourse.bass as bass
    return bass.AP(tensor=ap.tensor, offset=ap.offset,
                   ap=[list(ap.ap[0]), [0, n], list(ap.ap[1])])


def _view(tile_ap, start, dims):
    """View into a flat [128, N] AP at elem offset `start` with free dims
    [(stride, count), ...]."""
    import concourse.bass as bass
    return bass.AP(tensor=tile_ap.tensor, offset=tile_ap.offset + start,
                   ap=[list(tile_ap.ap[0])] + [list(d) for d in dims])


def _drive(*gens):
    """Round-robin drive generators to completion (interleaves PE work)."""
    gens = [g for g in gens if g is not None]
    while gens:
        nxt = []
        for g in gens:
            try:
                next(g)
                nxt.append(g)
            except StopIteration:
                pass
        gens = nxt


def _build_program():
    if "nc" in _CACHE:
        return _CACHE["nc"]

    import concourse.bacc as bacc
    import concourse.tile as tile
    import concourse.mybir as mybir

    f32 = mybir.dt.float32
    f16 = mybir.dt.float16
    fp8 = mybir.dt.float8e4
    AF = mybir.ActivationFunctionType
    ALU = mybir.AluOpType
    DR = mybir.MatmulPerfMode.DoubleRow

    nc = bacc.Bacc("TRN2", target_bir_lowering=False, debug=False)

    # x planes: [batch of 4 tiles, c-in-part 128, tile, chunk, token]
    xh_d = nc.dram_tensor("xh", [4, 128, 4, NCH, 128], fp8,
                          kind="ExternalInput")
    xl_d = nc.dram_tensor("xl", [4, 128, 4, NCH, 128], fp8,
                          kind="ExternalInput")
    wh_d = nc.dram_tensor("wh", [128, NCH, 3 * HD], fp8, kind="ExternalInput")
    wl_d = nc.dram_tensor("wl", [128, NCH, 3 * HD], fp8, kind="ExternalInput")
    ve_d = nc.dram_tensor("veN", [128, NT, HD], f16, kind="ExternalInput")
    cw_d = nc.dram_tensor("cwT", [HD, D], f16, kind="ExternalInput")
    # packed consts per partition: cos(512) | sin(512) | tri(128) | idn(128)
    cst_d = nc.dram_tensor("cst", [128, 1280], f16, kind="ExternalInput")
    out_d = nc.dram_tensor("outT", [D, T], f16, kind="ExternalOutput")

    with tile.TileContext(nc) as tc:
        with tc.tile_pool(name="const", bufs=1) as cpool, \
             tc.tile_pool(name="work", bufs=1) as wpool, \
             tc.tile_pool(name="xs", bufs=1) as xpool:
            # ---- resident inputs ----
            wh_sb = cpool.tile([128, NCH, 3 * HD], fp8)
            wl_sb = cpool.tile([128, NCH, 3 * HD], fp8)
            ve_sb = cpool.tile([128, NT, HD], f16)
            cw_sb = cpool.tile([HD, D], f16)
            cst_sb = cpool.tile([128, 1280], f16)
            ones_sb = cpool.tile([128, 128], f16)
            bexp_sb = cpool.tile([128, 1], f32)   # exp shift bias

            cstf = cst_sb[:]

            def cos_g(g):     # [128, 4, 32] for tile group g
                return _view(cstf, 128 * g, [[32, 4], [1, 32]])

            def sin_g(g):
                return _view(cstf, 512 + 128 * g, [[32, 4], [1, 32]])

            tri_v = _view(cstf, 1024, [[1, 128]])
            idn_v = _view(cstf, 1152, [[1, 128]])

            wu_sb = cpool.tile([128, 128], f16)   # PE warm-up operand
            nc.vector.memset(ones_sb[:], 1.0)
            nc.vector.memset(bexp_sb[:], -ESH)
            nc.vector.memset(wu_sb[:], 0.0)

            # ---- working buffers ----
            qkv = wpool.tile([128, NT, 3 * HD], f16)      # natural qkv
            sqs = wpool.tile([128, HD], f16)              # square scratch out
            rs = wpool.tile([128, NT, 2], f32)            # rms scalars q,k
            rx = wpool.tile([128, NT, 2], f32)            # rsqrt arg scratch
            rt = wpool.tile([128, NT, 2], f32)            # newton scratch
            qT_sb = wpool.tile([128, T], f16)             # q-hat.T [d, t]
            kT_sb = wpool.tile([128, NT, 128], f16)       # k-hat.T [d, si, s']
            pT_bufs = [wpool.tile([128, NT, TJ], f16, tag=f"pT{i}",
                                  name=f"pT{i}") for i in range(4)]

            # ============ phase 1: qkv (fp8 DoubleRow) + rms + rotary ======
            pools = {}
            if True:

                def dma_order_for_group(g):
                    # interleave remaining loads so x batches stay critical
                    if g < 3:
                        gs4 = slice(4 * (g + 1), 4 * (g + 2))
                        nc.sync.dma_start(ve_sb[:, gs4], ve_d[:, gs4])
                    if g == 1:
                        nc.sync.dma_start(cst_sb[:], cst_d[:])
                    elif g == 2:
                        nc.sync.dma_start(cw_sb[:], cw_d[:])

                def emit_qkv_mms(g, ps_tiles, xh_b, xl_b, term_major):
                    """24 DR matmuls per tile; term-major runs the 3 planes
                    as waves across all 4 tiles (x/w DMA arrival order)."""
                    terms = ((xh_b, wh_sb, NPAIR), (xl_b, wh_sb, NPAIR),
                             (xh_b, wl_sb, NPAIR))
                    i_mm = [0] * 4
                    n_mm = sum(2 * t[2] for t in terms)

                    def one(j, t_idx, P, half):
                        xa, wa, _ = terms[t_idx]
                        cs = slice(2 * P, 2 * P + 2)
                        hs2 = slice(192 * half, 192 * (half + 1))
                        nc.tensor.matmul(
                            ps_tiles[j][:, hs2],
                            xa[:, j, cs, :],
                            wa[:, cs, hs2],
                            start=(i_mm[j] == 0),
                            stop=(i_mm[j] == n_mm - 1),
                            perf_mode=DR,
                        )
                        i_mm[j] += 1

                    if term_major:
                        for t_idx in range(3):
                            for j in range(4):
                                for P in range(terms[t_idx][2]):
                                    for half in range(2):
                                        one(j, t_idx, P, half)
                    else:
                        for j in range(4):
                            for t_idx in range(3):
                                for P in range(terms[t_idx][2]):
                                    for half in range(2):
                                        one(j, t_idx, P, half)

                def emit_qkv_group(g):
                    gs = slice(4 * g, 4 * (g + 1))
                    xh_b = xpool.tile([128, 4, NCH, 128], fp8,
                                      tag=f"xh{g % 2}", name=f"xh_b{g}")
                    xl_b = xpool.tile([128, 4, NCH, 128], fp8,
                                      tag=f"xl{g % 2}", name=f"xl_b{g}")
                    if g == 0:
                        # critical order: first matmul wave needs xh+wh only
                        nc.sync.dma_start(xh_b[:], xh_d[g])
                        nc.sync.dma_start(wh_sb[:], wh_d[:])
                        nc.sync.dma_start(xl_b[:], xl_d[g])
                        nc.sync.dma_start(wl_sb[:], wl_d[:])
                        nc.sync.dma_start(ve_sb[:, 0:4], ve_d[:, 0:4])
                    else:
                        nc.sync.dma_start(xh_b[:], xh_d[g])
                        nc.sync.dma_start(xl_b[:], xl_d[g])
                    if g == 0:
                        # warm-up mms against a zero tile: anchor the PE
                        # p-state ramp during the initial x/w DMA gate so
                        # real matmuls run at full clock from the start
                        wps = pools["pq"].tile([128, 128], f32, tag="qkvp",
                                               name="wu_ps")
                        for _ in range(40):
                            nc.tensor.matmul(wps[:], wu_sb[:], wu_sb[:],
                                             start=True, stop=True)
                    ps_tiles = [pools["pq"].tile([128, 3 * HD], f32, tag="qkvp",
                                        name=f"ps{g}_{j}") for j in range(4)]
                    emit_qkv_mms(g, ps_tiles, xh_b, xl_b, term_major=(g == 0))
                    dma_order_for_group(g)
                    for j in range(4):
                        ti = 4 * g + j
                        ps = ps_tiles[j]
                        # qk drain f32->f16 with 1/WS descale (ACT)
                        nc.scalar.activation(qkv[:, ti, 0:2 * HD],
                                             ps[:, 0:2 * HD], AF.Copy,
                                             bias=0.0, scale=1.0 / WS)
                        # fused square+sum on DVE (STT with accum_out):
                        # rs[:,ti,c] = sum(q*q), f16 2x mode
                        nc.vector.scalar_tensor_tensor(
                            sqs[:], qkv[:, ti, 0:HD], 1.0,
                            qkv[:, ti, 0:HD], op0=ALU.bypass, op1=ALU.mult,
                            accum_out=rs[:, ti, 0:1])
                        nc.vector.scalar_tensor_tensor(
                            sqs[:], qkv[:, ti, HD:2 * HD], 1.0,
                            qkv[:, ti, HD:2 * HD], op0=ALU.bypass,
                            op1=ALU.mult, accum_out=rs[:, ti, 1:2])
                    # v drain fused with ve add (ve prescaled by lambda1;
                    # lambda0 folded into W plane scale): v = ps_v/WS + ve.
                    # Emitted promptly so the qkv PSUM banks release early
                    # (phase-2 score tiles reuse them).
                    for j in range(4):
                        ti = 4 * g + j
                        nc.vector.scalar_tensor_tensor(
                            qkv[:, ti, 2 * HD:], ps_tiles[j][:, 2 * HD:],
                            1.0 / WS, ve_sb[:, ti, :],
                            op0=ALU.mult, op1=ALU.add)
                    # rq = SCALE/sqrt(ms+eps), rk = 1/sqrt(ms+eps), computed
                    # as rsqrt on DVE (bit-trick seed + 2 Newton steps) so
                    # ACT never needs the sqrt table (one act-table load for
                    # the whole program).
                    xq = rx[:, gs, 0]
                    xk = rx[:, gs, 1]
                    nc.vector.tensor_scalar(xq, rs[:, gs, 0],
                                            1.0 / (HD * SCALE * SCALE),
                                            EPS / (SCALE * SCALE),
                                            op0=ALU.mult, op1=ALU.add)
                    nc.vector.tensor_scalar(xk, rs[:, gs, 1], 1.0 / HD,
                                            EPS, op0=ALU.mult, op1=ALU.add)
                    xz = rx[:, gs, :]
                    z = rs[:, gs, :]
                    t2 = rt[:, gs, :]
                    i32 = mybir.dt.int32
                    # seed: y0 = bits(0x5f3759df - (bits(x) >> 1));
                    # tensor_scalar is DVE-only, TT muls ride on Pool
                    nc.vector.tensor_scalar(z.bitcast(i32), xz.bitcast(i32),
                                            1, None,
                                            op0=ALU.arith_shift_right)
                    nc.vector.tensor_scalar(z.bitcast(i32), z.bitcast(i32),
                                            -1, None,
                                            op0=ALU.bitwise_xor)
                    nc.vector.tensor_scalar(z.bitcast(i32), z.bitcast(i32),
                                            0x5f3759df + 1, None,
                                            op0=ALU.add)
                    for _ in range(2):   # newton: y *= 1.5 - 0.5*x*y*y
                        nc.vector.tensor_mul(t2, z, z)
                        nc.vector.tensor_mul(t2, t2, xz)
                        nc.vector.tensor_scalar(t2, t2, -0.5, 1.5,
                                                op0=ALU.mult, op1=ALU.add)
                        nc.vector.tensor_mul(z, z, t2)

                def emit_tr_group(g):
                    hs = slice(4 * g, 4 * (g + 1))
                    for base in (0, HD):
                        eng = nc.vector if base == 0 else nc.gpsimd
                        x1 = qkv[:, hs, base + 0:base + 32]
                        x2 = qkv[:, hs, base + 64:base + 96]
                        t1 = wpool.tile([128, 4, 32], f16, tag=f"rot1{base}",
                                        name=f"t1_{base}")
                        t2 = wpool.tile([128, 4, 32], f16, tag=f"rot2{base}",
                                        name=f"t2_{base}")
                        t3 = wpool.tile([128, 4, 32], f16, tag=f"rot3{base}",
                                        name=f"t3_{base}")
                        t4 = wpool.tile([128, 4, 32], f16, tag=f"rot4{base}",
                                        name=f"t4_{base}")
                        eng.tensor_mul(t1[:], x1, cos_g(g))
                        eng.tensor_mul(t2[:], x2, sin_g(g))
                        eng.tensor_mul(t3[:], x2, cos_g(g))
                        eng.tensor_mul(t4[:], x1, sin_g(g))
                        eng.tensor_add(x1, t1[:], t2[:])
                        eng.tensor_sub(x2, t3[:], t4[:])
                    # normalize in place: q *= rq[t], k *= rk[t] (per-token
                    # scalar AP exempt from the DVE 2x-mode dtype check)
                    for j in range(4):
                        ti = 4 * g + j
                        nc.vector.tensor_scalar_mul(qkv[:, ti, 0:HD],
                                                    qkv[:, ti, 0:HD],
                                                    rs[:, ti, 0:1])
                        nc.vector.tensor_scalar_mul(qkv[:, ti, HD:2 * HD],
                                                    qkv[:, ti, HD:2 * HD],
                                                    rs[:, ti, 1:2])
                    # plain transposes (PE), drains on Pool
                    for base, isq in ((0, True), (HD, False)):
                        tp = pools["ptr"].tile([128, 4, 128], f32, tag="trp")
                        for j in range(4):
                            ti = 4 * g + j
                            nc.tensor.matmul(
                                tp[:, j, :],
                                qkv[:, ti, base:base + HD],
                                idn_v, start=True, stop=True)
                        dview = qT_sb[:, 512 * g:512 * (g + 1)] if isq \
                            else kT_sb[:, 4 * g:4 * (g + 1), :] \
                            .rearrange("p a b -> p (a b)")
                        tpf = tp[:].rearrange("p a b -> p (a b)")
                        # PSUM drains are ACT/DVE only (gpsimd can't)
                        if isq == (g % 2 == 0):
                            nc.scalar.copy(dview, tpf)
                        else:
                            nc.vector.tensor_copy(dview, tpf)

                def act_copy(dst, src):
                    nc.scalar.copy(dst, src)

                def dve_copy(dst, src):
                    nc.vector.tensor_copy(dst, src)

                # PSUM drains are ACT/DVE only (gpsimd can't touch PSUM).
                # Early c_proj blocks drain DVE-heavy (ACT busy with exps);
                # late blocks ACT-heavy (exps done by then).
                drain_early = [dve_copy, act_copy, dve_copy, act_copy,
                               dve_copy, act_copy, dve_copy, act_copy]
                drain_late = [act_copy, dve_copy, act_copy, act_copy,
                              dve_copy, act_copy, act_copy, dve_copy]

                def gen_scores(tj, psc, pw=2):
                    pT_sb = pT_bufs[tj]
                    t0 = TJ * tj
                    n_act = 4 * (tj + 1)
                    for p in range(n_act // pw):
                        sc2 = psc.tile([128, pw, TJ], f32, tag="scp",
                                       name=f"sc2_{tj}_{p}")
                        # diagonal blocks compute their full row too (the
                        # sub-diagonal part is finite garbage, never read by
                        # dn/yT) so the paired exp reads fully-written PSUM
                        for b in range(pw):
                            si = pw * p + b
                            nc.tensor.matmul(
                                sc2[:, b, :], kT_sb[:, si, :],
                                qT_sb[:, t0:t0 + TJ],
                                start=True, stop=True)
                            yield
                        # paired exp over both banks (garbage regions of
                        # diagonal blocks are exp'd too but never read)
                        nc.scalar.activation(
                            pT_sb[:, pw * p:pw * p + pw, :], sc2[:], AF.Exp,
                            bias=bexp_sb[:])
                    # merged diagonal triangle mask (DVE)
                    import concourse.bass as bass
                    base = pT_sb[:, 4 * tj, 0:128]
                    mview = bass.AP(
                        tensor=base.tensor, offset=base.offset,
                        ap=[list(base.ap[0]), [TJ + 128, 4], [1, 128]])
                    tri_b = _bcast_mid(tri_v, 4)
                    # SBUF-only -> Pool (keeps ACT/DVE free for drains)
                    nc.gpsimd.tensor_tensor(mview, mview, tri_b, op=ALU.mult)

                def gen_av(tj, out_yh, pav, fin):
                    """dn+yT per 256-wide half, sharing one PSUM bank
                    ([128, 2, 256] f32), double-buffered across halves.
                    For the two big t-blocks, adjacent full probability
                    blocks are pre-summed on DVE so the denominator needs
                    one PE matmul per pair."""
                    pT_sb = pT_bufs[tj]
                    n_act = 4 * (tj + 1)
                    npair = (4 * tj) // 2 if tj >= 1 else 0
                    if npair:
                        pS = wpool.tile([128, 6, TJ], f16, tag="pSum",
                                        name=f"pS_{tj}")
                        for p in range(npair):
                            nc.vector.tensor_add(pS[:, p, :],
                                                 pT_sb[:, 2 * p, :],
                                                 pT_sb[:, 2 * p + 1, :])
                    yh = fin.tile([128, TJ], f16, tag=f"yh{tj % 2}",
                                  name=f"yh_{tj}")
                    chunks = ((0, 256), (256, 256))
                    for h, (t0h, tl) in enumerate(chunks):
                        dy = pav.tile([128, 2, 256], f32, tag="dy",
                                      name=f"dy_{tj}_{h}")
                        act = [si for si in range(n_act)
                               if (128 * (si - 4 * tj)
                                   if si > 4 * tj else 0) < t0h + tl]
                        # dn+yT share one PSUM bank: ONE accumulation group
                        # (start on the first mm, stop on the last; first
                        # write per element overwrites, later ones add).
                        # dn ops: paired sums for si < 2*npair, then singles.
                        dn_ops = [("p", p) for p in range(npair)] + \
                                 [("s", si) for si in act
                                  if si >= 2 * npair]
                        n_ops = len(dn_ops) + len(act)
                        iop = 0
                        for kind, ix in dn_ops:
                            if kind == "p":
                                srcap = pS[:, ix, t0h:t0h + tl]
                                s0 = 0
                            else:
                                o = ix - 4 * tj
                                off = 128 * o if o > 0 else 0
                                s0 = max(off - t0h, 0)
                                srcap = pT_sb[:, ix, t0h + s0:t0h + tl]
                            nc.tensor.matmul(
                                dy[:, 0, s0:tl], ones_sb[:], srcap,
                                start=(iop == 0), stop=False)
                            iop += 1
                            yield
                        for idx, si in enumerate(act):
                            o = si - 4 * tj
                            off = 128 * o if o > 0 else 0
                            s0 = max(off - t0h, 0)
                            nc.tensor.matmul(
                                dy[:, 1, s0:tl], qkv[:, si, 2 * HD:],
                                pT_sb[:, si, t0h + s0:t0h + tl],
                                start=False, stop=(idx == len(act) - 1))
                            yield
                        rdn = fin.tile([128, 256], f32, tag=f"rdn{h % 2}",
                                       name=f"rdn_{tj}_{h}")
                        nc.vector.reciprocal(rdn[:, 0:tl], dy[:, 0, 0:tl])
                        nc.vector.tensor_mul(yh[:, t0h:t0h + tl],
                                             dy[:, 1, 0:tl], rdn[:, 0:tl])
                    out_yh.append(yh)

                def gen_cproj(tj, yh, po, stp, po2=None, po3=None):
                    import concourse.bass as bass
                    st = stp.tile([128, 8, TJ], f16, tag=f"st{tj % 2}",
                                  name=f"st_{tj}")
                    halves = ((0, 256), (256, 256)) \
                        if tj == 3 else ((0, TJ),)
                    e_per = 4
                    for (t0h, tl) in halves:
                        t0 = TJ * tj + t0h
                        for e in range(8):
                            # tail cycles three pools (psc2 is idle by then)
                            # so the mms aren't PSUM-bank starved
                            if po3 is not None and e % 3 == 2:
                                pot = po3.tile([128, 2, tl], f32, tag="scp",
                                               name=f"pot_{tj}_{t0h}_{e}")
                                pot = pot[:, 0, :]
                            elif po2 is not None and e % 3 == 1:
                                pot = po2.tile([128, tl], f32, tag="dy",
                                               name=f"pot_{tj}_{t0h}_{e}")
                                pot = pot[:]
                            else:
                                pot = po.tile([128, tl], f32, tag="pot",
                                              name=f"pot_{tj}_{t0h}_{e}")
                            pap = pot if not hasattr(pot, 'tile') else pot
                            try:
                                pap = pot[:]
                            except Exception:
                                pap = pot
                            nc.tensor.matmul(
                                pap, cw_sb[:, 128 * e:128 * (e + 1)],
                                yh[:, t0h:t0h + tl],
                                start=True, stop=True)
                            yield
                            (drain_early if tj < 2 else drain_late)[e](st[:, e, t0h:t0h + tl], pap)
                            if e % e_per == e_per - 1:
                                e0 = e - e_per + 1
                                dst = out_d[0:128, t0:t0 + tl]
                                dstb = bass.AP(
                                    tensor=dst.tensor,
                                    offset=dst.offset + 128 * e0 * T,
                                    ap=[list(dst.ap[0]), [128 * T, e_per],
                                        list(dst.ap[1])])
                                nc.sync.dma_start(
                                    dstb, st[:, e0:e0 + e_per, t0h:t0h + tl])

                # ==== emission ====
                # phase 1 with scores(0..1) overlapped: scores(tj) only
                # needs transpose groups <= tj, so they fill PE's DMA-wait
                # gaps. PSUM: pq(4) + ptr(2) + psc1(2 banks, bufs=1) = 8.
                with tc.tile_pool(name="ps_qkv", bufs=4, space="PSUM") as pq_, \
                     tc.tile_pool(name="ps_tr", bufs=2, space="PSUM") as ptr_, \
                     tc.tile_pool(name="ps_sc1", bufs=2, space="PSUM") as psc1:
                    pools["pq"] = pq_
                    pools["ptr"] = ptr_
                    for g in range(4):
                        emit_qkv_group(g)
                        if g >= 1:
                            emit_tr_group(g - 1)
                            if g - 1 <= 1:
                                _drive(gen_scores(g - 1, psc1, pw=1))
                    emit_tr_group(3)

                # phase 2: scores(2..3) + av/cproj; the freed phase-1 banks
                # become psc2(4) + pdn(1) + py(1) + po(2) = 8.
                with tc.tile_pool(name="ps_sc2", bufs=2, space="PSUM") as psc2, \
                     tc.tile_pool(name="ps_av", bufs=2, space="PSUM") as pav, \
                     tc.tile_pool(name="ps_o", bufs=2, space="PSUM") as po, \
                     tc.tile_pool(name="fin", bufs=2) as fin, \
                     tc.tile_pool(name="stp", bufs=1) as stp:
                    # tj=0 (the smallest attention block) is processed LAST
                    # so the terminal av->yh->cproj->store chain is short
                    yhs = {}

                    def av(tj):
                        out = []
                        g = gen_av(tj, out, pav, fin)
                        yhs[tj] = lambda: out[0]
                        return g

                    def cproj(tj, **kw):
                        return gen_cproj(tj, yhs[tj](), po, stp, **kw)

                    _drive(gen_scores(2, psc2), av(0))
                    _drive(gen_scores(3, psc2), av(1), cproj(0))
                    _drive(av(2), cproj(1))
                    _drive(av(3), cproj(2))
                    _drive(cproj(3, po2=pav, po3=psc2))

    nc.compile()
    _CACHE["nc"] = nc
    return nc


def _host_inputs(x, ve, qkv_w, lambdas, c_proj_w):
    """Build the 8 per-core input maps (layout transforms only)."""
    import ml_dtypes
    f16 = ml_dtypes.float16 if hasattr(ml_dtypes, "float16") else np.float16
    e4 = ml_dtypes.float8_e4m3
    x = np.asarray(x, np.float32)
    ve = np.asarray(ve, np.float32)
    qkv_w = np.asarray(qkv_w, np.float32)
    lambdas = np.asarray(lambdas, np.float32)
    c_proj_w = np.asarray(c_proj_w, np.float32)

    # x planes: [4 batch, 128 cin, 4 tile, NCH, 128 tok]
    xr = x[0].reshape(NT, 128, NCH, 128).transpose(0, 3, 2, 1)
    xr = xr.reshape(4, 4, 128, NCH, 128).transpose(0, 2, 1, 3, 4)
    xh = xr.astype(e4)
    xl = (xr - xh.astype(np.float32)).astype(e4)
    xh = np.ascontiguousarray(xh)
    xl = np.ascontiguousarray(xl)

    freq = (1.0 / 1024.0) ** np.linspace(0.0, 1.0, HD // 4, dtype=np.float32)
    theta = np.arange(T, dtype=np.float32)[:, None] * freq[None, :]  # [T, 32]
    cosT = np.cos(theta).astype(f16).reshape(NT, 128, 32).transpose(1, 0, 2)
    sinT = np.sin(theta).astype(f16).reshape(NT, 128, 32).transpose(1, 0, 2)
    tri = (np.arange(128)[None, :] >= np.arange(128)[:, None]).astype(f16)
    idn = np.eye(128, dtype=np.float32).astype(f16)
    cst = np.ascontiguousarray(np.concatenate([
        cosT.reshape(128, 512), sinT.reshape(128, 512), tri, idn,
    ], axis=1))  # [128, 1280]

    lam0, lam1 = float(lambdas[0]), float(lambdas[1])
    wscale = np.concatenate([np.full(2 * HD, WS, np.float32),
                             np.full(HD, WS * lam0, np.float32)])

    in_maps = []
    for h in range(NCORES):
        sl = slice(128 * h, 128 * (h + 1))
        # W planes: [128 cin, NCH, 3*HD], prescaled
        whd = qkv_w[:, sl, :]                          # [3, 128, 1024]
        wt = whd.transpose(2, 0, 1).reshape(D, 3 * HD)  # [cin-full, 384]
        wt = wt * wscale[None, :]
        wt = wt.reshape(NCH, 128, 3 * HD).transpose(1, 0, 2)  # [128, NCH, 384]
        wh = wt.astype(e4)
        wl = (wt - wh.astype(np.float32)).astype(e4)
        wh = np.ascontiguousarray(wh)
        wl = np.ascontiguousarray(wl)
        veh = np.ascontiguousarray(
            (lam1 * ve[0, :, sl]).reshape(NT, 128, HD)
            .transpose(1, 0, 2).astype(f16))
        cwh = np.ascontiguousarray(c_proj_w[:, sl].T.astype(f16))  # [128, 1024]
        in_maps.append({
            "xh": xh, "xl": xl, "wh": wh, "wl": wl, "veN": veh, "cwT": cwh,
            "cst": cst,
        })
    return in_maps


def run(x, ve, qkv_w, lambdas, c_proj_w, trace=False):
    from concourse.bass_utils import run_bass_kernel_spmd

    nc = _build_program()
    in_maps = _host_inputs(x, ve, qkv_w, lambdas, c_proj_w)
    res = run_bass_kernel_spmd(
        nc, in_maps, core_ids=list(range(NCORES)), trace=trace)
    acc = np.zeros((D, T), np.float64)
    for r in res.results:
        acc += r["outT"].astype(np.float64)
    out = acc.astype(np.float32).T.reshape(B, T, D)
    return out, res


def kernel(x, ve, qkv_w, lambdas, c_proj_w):
    out, _ = run(x, ve, qkv_w, lambdas, c_proj_w, trace=False)
    return out
